# revision 16
# baseline (speedup 1.0000x reference)
"""Trainium2 Bass kernel for WeightedSignedConv (first_aggr=True) GCN block.

Strategy (8 NeuronCores, one SPMD program):
  - 50000 dst nodes are padded to 50176 = 392 tiles of 128; tiles are
    sorted by edge count and dealt to (core, slot) so all 8 cores see
    nearly identical work per slot (one shared program fits all cores).
  - Host-side: edges are bucketed by (dst chunk of 4 tiles, sign(edge_attr));
    the 1/count normalization is folded into per-edge weights. Within a
    bucket, edges sharing a src PAIR share one gathered slot (pair-dedup,
    ~4% fewer descriptors); same-(pair,parity) extras overflow to their own
    slots. Slots are sorted [even-only | both | odd-only] so two thresholds
    per bucket decide which 128-slot blocks need which parity passes.
  - The gather uses int16 indices (idx = src>>1 < 25088), fetching bf16
    PAIR rows (512B/descriptor) via gpsimd.dma_gather in 1024-idx calls.
    SWDGE descriptor generation (~8ns/idx, Q7-ucode serial) is the
    bottleneck; everything else is hidden under it.
  - Device-side per core: build one-hot scatter matrices
    S[slot, d] = w * (enc(iota) == enc(dloc)) over 512-dst PSUM windows.
    iota/dloc use an injective bf16-exact encoding of [0,512) (identity /
    even / mult-of-4 ranges) since bf16 integers are exact only to 256.
    S-builds are split 3:2 between the Vector engine (fused
    is_equal+mult tensor_scalar) and the Scalar engine (Square+Relu trick:
    w*relu(1-(iota-dloc)^2)), because GPSIMD descriptor generation
    contends with the DVE for an SBUF port and halves its throughput.
  - Aggregation: aggT[f, d] += Xg[slot, f]^T S[slot, d] on the tensor
    engine in PSUM (bf16 operands, fp32 accumulate), reading the even or
    odd half of each gathered pair; wrong-parity weights are 0.
    Projection: out^T[o, d] = W_l^T agg + W_r^T x^T (bf16), finished with
    fused ReLU+bias on the Scalar engine in fp32.
  - Output is produced transposed ([256, D_core] per core); the host
    transposes/reorders, which is pure layout assembly.

Measured: ~830us on 8 trn2 NeuronCores (gather-bound), rel err ~3e-3.
"""

import numpy as np

P = 128
NCORES = 8
CHUNK_TILES = 4          # dst tiles per PSUM window (512 dsts = 1 bank)
MSG_DT_NAME = "bfloat16"  # gathered x + S dtype
PROJ_DT_NAME = "bfloat16"  # projection matmul dtype
GATHER_MAX = 1024        # max idxs per dma_gather call


def _ceil_div(a, b):
    return (a + b - 1) // b


def _enc_bf16(v):
    """Injective map [0,512) -> bf16-exact floats (id, even, mult-of-4)."""
    v = np.asarray(v, dtype=np.float64)
    return np.where(v < 256, v, np.where(v < 384, 2 * v - 256, 4 * v - 1024))


def _preprocess(x, src, dst, attr, slots_per_core, msg_np):
    """Bucket/pad edges; build per-core device arrays + block metadata."""
    n, f = x.shape
    assert f == P
    tiles_total = NCORES * slots_per_core
    n_pad = tiles_total * P

    pos = attr > 0
    neg = attr < 0
    keep = pos | neg
    absa = np.abs(attr)
    cntp = np.bincount(dst[pos], minlength=n).astype(np.float32)
    cntn = np.bincount(dst[neg], minlength=n).astype(np.float32)
    recp = 1.0 / np.maximum(cntp, 1.0)
    recn = 1.0 / np.maximum(cntn, 1.0)
    w1_all = absa.astype(np.float32) * np.where(pos, recp[dst], recn[dst])

    s_ = src[keep].astype(np.int64)
    d_ = dst[keep].astype(np.int64)
    sg = np.where(pos[keep], 0, 1).astype(np.int64)
    w1 = w1_all[keep].astype(np.float32)
    pairidx = s_ >> 1
    parity = s_ & 1

    tile_g = d_ // P

    # Dealing: the program is shared across cores, so each (chunk, sign)
    # bucket is padded to the max slot count over the 8 cores. Try several
    # tile->core dealing strategies and keep the one with the least padding
    # (evaluated on actual deduped slot counts).
    tile_edges = np.bincount(tile_g, minlength=tiles_total)
    tpos = np.bincount(tile_g[sg == 0], minlength=tiles_total)
    tneg = np.bincount(tile_g[sg == 1], minlength=tiles_total)
    order_t = np.argsort(-tile_edges, kind="stable")
    n_chunks_ = _ceil_div(slots_per_core, CHUNK_TILES)

    def _deal_rank():
        rank = np.argsort(np.argsort(-tile_edges))
        return rank % NCORES, rank // NCORES

    def _deal_greedy(per_sign):
        tc = np.empty(tiles_total, dtype=np.int64)
        ts = np.empty(tiles_total, dtype=np.int64)
        cum = np.zeros((NCORES, 2), dtype=np.int64)
        for r in range(slots_per_core):
            if r % CHUNK_TILES == 0:
                cum[:] = 0
            row = order_t[r * NCORES : (r + 1) * NCORES]
            rowtiles = row[np.argsort(-(tpos[row] + tneg[row]),
                                      kind="stable")]
            free = list(range(NCORES))
            for t in rowtiles:
                if per_sign:
                    cost = [max(cum[c, 0] + tpos[t], cum[c, 1] + tneg[t])
                            for c in free]
                else:
                    cost = [cum[c, 0] + cum[c, 1] for c in free]
                c = free.pop(int(np.argmin(cost)))
                tc[t] = c
                ts[t] = r
                cum[c, 0] += tpos[t]
                cum[c, 1] += tneg[t]
        return tc, ts

    def _eval_npad(tc, ts):
        co = tc[tile_g]
        sl = ts[tile_g]
        ch = sl // CHUNK_TILES
        ky = (co * n_chunks_ + ch) * 2 + sg
        nk = NCORES * n_chunks_ * 2
        kq = ky * (1 << 15) + pairidx
        kqs = np.sort(kq)
        newp = np.ones(kqs.size, bool)
        newp[1:] = kqs[1:] != kqs[:-1]
        up = np.bincount((kqs >> 15)[newp], minlength=nk)
        kqp = np.sort(kq * 2 + parity)
        newpp = np.ones(kqp.size, bool)
        newpp[1:] = kqp[1:] != kqp[:-1]
        upp = np.bincount((kqp >> 16)[newpp], minlength=nk)
        epk = np.bincount(ky, minlength=nk)
        spk = (up + (epk - upp)).reshape(NCORES, n_chunks_, 2)
        return int((_ceil_div(spk.max(axis=0), P) * P).sum())

    cands = [_deal_rank(), _deal_greedy(False), _deal_greedy(True)]
    scores = [_eval_npad(tc, ts) for tc, ts in cands]
    tile_core, tile_slot = cands[int(np.argmin(scores))]

    core = tile_core[tile_g]
    slot = tile_slot[tile_g]
    chunk = slot // CHUNK_TILES
    dloc = (slot % CHUNK_TILES) * P + d_ % P

    n_chunks = _ceil_div(slots_per_core, CHUNK_TILES)

    # group key: (core, chunk, sign); within a bucket, edges sharing a
    # src PAIR share one gathered slot (h-passes read even/odd halves), so
    # each pair is fetched once per bucket. Extra edges with the same
    # (pair, parity) overflow into their own slots. Slots are sorted by
    # class [E=h0-only | M=both | O=h1-only] so two thresholds (p0, p1)
    # still decide which blocks need which halves.
    key = (core * n_chunks + chunk) * 2 + sg
    nkeys = NCORES * n_chunks * 2

    ordK = np.lexsort((parity, pairidx, key))
    k_s = key[ordK]
    q_s = pairidx[ordK]
    par_s = parity[ordK]
    dl_s = dloc[ordK]
    w_s = w1[ordK]
    new_pair = np.ones(k_s.size, dtype=bool)
    new_pair[1:] = (k_s[1:] != k_s[:-1]) | (q_s[1:] != q_s[:-1])
    new_pp = new_pair.copy()
    new_pp[1:] |= par_s[1:] != par_s[:-1]
    grp = np.cumsum(new_pair) - 1
    n_grp = int(new_pair.sum())

    g_key = k_s[new_pair]
    g_pair = q_s[new_pair]
    g_d = np.zeros((n_grp, 2), dtype=np.float64)
    g_w = np.zeros((n_grp, 2), dtype=np.float64)
    g_has = np.zeros((n_grp, 2), dtype=bool)
    for h in (0, 1):
        m = new_pp & (par_s == h)
        g_d[grp[m], h] = dl_s[m]
        g_w[grp[m], h] = w_s[m]
        g_has[grp[m], h] = True
    g_class = np.where(g_has[:, 0] & g_has[:, 1], 1,
                       np.where(g_has[:, 0], 0, 2))

    ov = ~new_pp
    o_key = k_s[ov]
    o_pair = q_s[ov]
    o_par = par_s[ov]
    o_d = np.zeros((o_key.size, 2), dtype=np.float64)
    o_w = np.zeros((o_key.size, 2), dtype=np.float64)
    o_d[np.arange(o_key.size), o_par] = dl_s[ov]
    o_w[np.arange(o_key.size), o_par] = w_s[ov]
    o_class = np.where(o_par == 0, 0, 2)

    s_key = np.concatenate([g_key, o_key])
    s_pair = np.concatenate([g_pair, o_pair])
    s_class = np.concatenate([g_class, o_class])
    s_d = np.concatenate([g_d, o_d])
    s_w = np.concatenate([g_w, o_w])

    cnt_kc = np.bincount(s_key * 3 + s_class, minlength=nkeys * 3).reshape(
        NCORES, n_chunks, 2, 3
    )
    nE = cnt_kc[..., 0]
    nM = cnt_kc[..., 1]
    nO = cnt_kc[..., 2]
    tot_kc = nE + nM + nO
    blocks = _ceil_div(tot_kc.max(axis=0), P)  # [chunk, sign]
    blocks = np.maximum(blocks, 1)
    p0 = nE.min(axis=0) // P                   # h1 passes for j >= p0
    p1 = np.minimum(_ceil_div((nE + nM).max(axis=0), P), blocks)

    # layout: per chunk: blocks of sign 0 then sign 1; per block a list of
    # (half, metacol) passes
    gstart = np.zeros((n_chunks, 2), dtype=np.int64)
    chunks = []  # (chunk_idx, width, chunk_block0, nb_chunk)
    windows = {}  # (chunk, sign) -> [(gblock, half, metacol), ...]
    b = 0
    mc = 0
    for c in range(n_chunks):
        cb0 = b
        for s in (0, 1):
            gstart[c, s] = b
            ops = []
            for j in range(int(blocks[c, s])):
                gb = b + j
                if j < p1[c, s]:
                    ops.append((gb, 0, mc))
                    mc += 1
                if j >= p0[c, s]:
                    ops.append((gb, 1, mc))
                    mc += 1
            windows[(c, s)] = ops
            b += int(blocks[c, s])
        w = min(CHUNK_TILES, slots_per_core - c * CHUNK_TILES) * P
        chunks.append((c, w, cb0, b - cb0))
    tot_blocks = b
    tot_cols = mc
    npad = tot_blocks * P

    # per-slot destination position in the padded per-core arrays
    ordS = np.lexsort((s_class, s_key))
    key_s2 = s_key[ordS]
    group_first = np.searchsorted(key_s2, np.arange(nkeys), side="left")
    rank_s = np.arange(key_s2.size) - group_first[key_s2]
    gstart_flat = gstart.reshape(-1)
    local_key = key_s2 % (n_chunks * 2)
    sslot = gstart_flat[local_key] * P + rank_s

    core_s2 = key_s2 // (n_chunks * 2)
    pair_s2 = s_pair[ordS]
    d_s2 = s_d[ordS]
    w_s2 = s_w[ordS]

    # block/half -> metacol lookup
    colmap = -np.ones((tot_blocks, 2), dtype=np.int64)
    for ops in windows.values():
        for gb, h, mcol in ops:
            colmap[gb, h] = mcol

    idx16_list, dw_list, ww_list = [], [], []
    for cc in range(NCORES):
        m = core_s2 == cc
        sp = np.zeros(npad, dtype=np.int64)
        dp = np.zeros((npad, 2), dtype=np.float64)
        wp = np.zeros((npad, 2), dtype=np.float64)
        sp[sslot[m]] = pair_s2[m]
        dp[sslot[m]] = d_s2[m]
        wp[sslot[m]] = w_s2[m]
        tmp = sp.reshape(-1, 16).T.astype(np.int16)
        idx16_list.append(np.tile(tmp, (8, 1)))
        dcols = np.zeros((P, tot_cols), dtype=np.float64)
        wcols = np.zeros((P, tot_cols), dtype=np.float64)
        dp2 = dp.reshape(-1, P, 2)
        wp2 = wp.reshape(-1, P, 2)
        for gb in range(tot_blocks):
            for h in (0, 1):
                mcol = colmap[gb, h]
                if mcol < 0:
                    continue
                dcols[:, mcol] = _enc_bf16(dp2[gb, :, h])
                wcols[:, mcol] = wp2[gb, :, h]
        dw_list.append(np.ascontiguousarray(dcols).astype(np.float32))
        ww_list.append(np.ascontiguousarray(wcols).astype(np.float32))

    meta = dict(
        n=n,
        n_pad=n_pad,
        slots_per_core=slots_per_core,
        n_chunks=n_chunks,
        tot_blocks=tot_blocks,
        tot_cols=tot_cols,
        npad=npad,
        chunks=chunks,
        windows=windows,
        tile_core=tile_core,
        tile_slot=tile_slot,
    )
    return meta, idx16_list, dw_list, ww_list


def _build_program(meta, msg_dt, proj_dt):
    import concourse.bacc as bacc
    import concourse.mybir as mybir
    import concourse.tile as tile

    f32 = mybir.dt.float32
    dcore = meta["slots_per_core"] * P
    wmax = CHUNK_TILES * P
    npairs = meta["n_pad"] // 2

    nc = bacc.Bacc(
        "TRN2", target_bir_lowering=False, debug=False, num_devices=NCORES,
    )
    xall = nc.dram_tensor("xall", [npairs, 2 * P], msg_dt,
                          kind="ExternalInput")
    idx16 = nc.dram_tensor(
        "idx16", [P, meta["npad"] // 16], mybir.dt.int16, kind="ExternalInput"
    )
    dlocd = nc.dram_tensor(
        "dloc", [P, meta["tot_cols"]], f32, kind="ExternalInput"
    )
    wpd = nc.dram_tensor(
        "wp", [P, meta["tot_cols"]], f32, kind="ExternalInput"
    )
    dlocnd = nc.dram_tensor(
        "dlocn", [P, meta["tot_cols"]], f32, kind="ExternalInput"
    )
    wnd = nc.dram_tensor(
        "wn", [P, meta["tot_cols"]], f32, kind="ExternalInput"
    )
    iotad = nc.dram_tensor("iota", [P, wmax], msg_dt, kind="ExternalInput")
    xTd = nc.dram_tensor("xT", [P, dcore], proj_dt, kind="ExternalInput")
    wd = {}
    for nm in ("wpl", "wpr", "wnl", "wnr"):
        wd[nm] = nc.dram_tensor(nm, [P, P], proj_dt, kind="ExternalInput")
    bd = {
        0: nc.dram_tensor("bpos", [P, 1], f32, kind="ExternalInput"),
        1: nc.dram_tensor("bneg", [P, 1], f32, kind="ExternalInput"),
    }
    outd = nc.dram_tensor("outT", [2 * P, dcore], f32, kind="ExternalOutput")

    # process chunks largest-first: the tail after the last gather is the
    # last chunk's compute chain, so make that chunk the smallest
    chunk_order = sorted(meta["chunks"], key=lambda c: -c[3])
    # idx columns for the first-processed chunk load in their own DMA so
    # the first gather doesn't wait on the whole index array
    f_cb0, f_nb = chunk_order[0][2], chunk_order[0][3]
    lo_cols, hi_cols = f_cb0 * 8, (f_cb0 + f_nb) * 8

    with tile.TileContext(nc) as tc:
        with tc.tile_pool(name="const", bufs=1) as cpool, \
             tc.tile_pool(name="work", bufs=4) as wpool, \
             tc.tile_pool(name="spool", bufs=10) as spool, \
             tc.tile_pool(name="psum", bufs=2, space="PSUM") as ppool:
            idx_t = cpool.tile([P, meta["npad"] // 16], mybir.dt.int16)
            dloc_t = cpool.tile([P, meta["tot_cols"]], f32)
            wp_t = cpool.tile([P, meta["tot_cols"]], f32)
            dlocn_t = cpool.tile([P, meta["tot_cols"]], f32)
            wn_t = cpool.tile([P, meta["tot_cols"]], f32)
            iota_t = cpool.tile([P, wmax], msg_dt)
            w_t = {nm: cpool.tile([P, P], proj_dt, name=f"w_{nm}",
                                  tag=f"w_{nm}") for nm in wd}
            b_t = {s: cpool.tile([P, 1], f32, name=f"b_{s}", tag=f"b_{s}")
                   for s in (0, 1)}
            nc.sync.dma_start(out=idx_t[:, lo_cols:hi_cols],
                              in_=idx16[:, lo_cols:hi_cols])
            if lo_cols > 0:
                nc.sync.dma_start(out=idx_t[:, :lo_cols],
                                  in_=idx16[:, :lo_cols])
            if hi_cols < meta["npad"] // 16:
                nc.sync.dma_start(out=idx_t[:, hi_cols:],
                                  in_=idx16[:, hi_cols:])
            nc.sync.dma_start(out=iota_t[:], in_=iotad[:])
            nc.sync.dma_start(out=dloc_t[:], in_=dlocd[:])
            nc.sync.dma_start(out=wp_t[:], in_=wpd[:])
            nc.sync.dma_start(out=dlocn_t[:], in_=dlocnd[:])
            nc.sync.dma_start(out=wn_t[:], in_=wnd[:])
            for nm in wd:
                nc.sync.dma_start(out=w_t[nm][:], in_=wd[nm][:])
            for s in (0, 1):
                nc.sync.dma_start(out=b_t[s][:], in_=bd[s][:])

            wl = {0: w_t["wpl"], 1: w_t["wnl"]}
            wr = {0: w_t["wpr"], 1: w_t["wnr"]}

            spass = 0
            for ci, w, cb0, nb_chunk in chunk_order:
                xg = wpool.tile([P, nb_chunk, 2 * P], msg_dt, name="xg",
                                tag="xg")
                done = 0
                while done < nb_chunk:
                    g = min(nb_chunk - done, GATHER_MAX // P)
                    gb0 = cb0 + done
                    nc.gpsimd.dma_gather(
                        out_ap=xg[:, done : done + g, :],
                        in_ap=xall[:],
                        idxs_ap=idx_t[:, gb0 * 8 : (gb0 + g) * 8],
                        num_idxs=g * P,
                        num_idxs_reg=g * P,
                        elem_size=2 * P,
                        single_packet=False,
                    )
                    done += g

                agg_ps = {
                    s: ppool.tile([P, w], f32, name=f"agg{s}", tag=f"agg{s}")
                    for s in (0, 1)
                }
                for s in (0, 1):
                    ops = meta["windows"][(ci, s)]
                    for j, (gb, h, mcol) in enumerate(ops):
                        s_t = spool.tile([P, w], msg_dt, name="S", tag="S")
                        if spass % 5 < 3:
                            nc.vector.tensor_scalar(
                                out=s_t[:],
                                in0=iota_t[:, :w],
                                scalar1=dloc_t[:, mcol : mcol + 1],
                                scalar2=wp_t[:, mcol : mcol + 1],
                                op0=mybir.AluOpType.is_equal,
                                op1=mybir.AluOpType.mult,
                            )
                        else:
                            z2 = spool.tile([P, w], msg_dt, name="Z", tag="Z")
                            nc.scalar.activation(
                                out=z2[:], in_=iota_t[:, :w],
                                func=mybir.ActivationFunctionType.Square,
                                bias=dlocn_t[:, mcol : mcol + 1],
                            )
                            nc.scalar.activation(
                                out=s_t[:], in_=z2[:],
                                func=mybir.ActivationFunctionType.Relu,
                                scale=wn_t[:, mcol : mcol + 1],
                                bias=wp_t[:, mcol : mcol + 1],
                            )
                        spass += 1
                        nc.tensor.matmul(
                            out=agg_ps[s][:],
                            lhsT=xg[:, gb - cb0, h * P : (h + 1) * P],
                            rhs=s_t[:],
                            start=(j == 0),
                            stop=(j == len(ops) - 1),
                        )

                xT_t = wpool.tile([P, w], proj_dt, name="xT", tag="xT")
                nc.sync.dma_start(
                    out=xT_t[:],
                    in_=xTd[:, ci * wmax : ci * wmax + w],
                )
                for s in (0, 1):
                    agg_sb = wpool.tile([P, w], proj_dt, name=f"aggsb{s}",
                                        tag=f"aggsb{s}")
                    nc.scalar.copy(out=agg_sb[:], in_=agg_ps[s][:])
                    out_ps = ppool.tile([P, w], f32, name=f"out{s}",
                                        tag=f"out{s}")
                    nc.tensor.matmul(
                        out=out_ps[:], lhsT=wl[s][:], rhs=agg_sb[:],
                        start=True, stop=False,
                    )
                    nc.tensor.matmul(
                        out=out_ps[:], lhsT=wr[s][:], rhs=xT_t[:],
                        start=False, stop=True,
                    )
                    out_sb = wpool.tile([P, w], f32, name=f"outsb{s}",
                                        tag=f"outsb{s}")
                    nc.scalar.activation(
                        out=out_sb[:], in_=out_ps[:],
                        func=mybir.ActivationFunctionType.Relu,
                        bias=b_t[s][:],
                    )
                    nc.sync.dma_start(
                        out=outd[s * P : (s + 1) * P,
                                 ci * wmax : ci * wmax + w],
                        in_=out_sb[:],
                    )
    nc.compile()
    return nc


def _run(x, edge_index, edge_attr, w_pos_l, w_pos_r, b_pos_r, w_neg_l,
         w_neg_r, b_neg_r, slots_per_core=49, sim=False, trace=False,
         trace_all=False):
    import concourse.mybir as mybir
    from concourse.bass_utils import run_bass_kernel_spmd

    msg_dt = getattr(mybir.dt, MSG_DT_NAME)
    proj_dt = getattr(mybir.dt, PROJ_DT_NAME)
    msg_np = np.dtype(mybir.dt.np(msg_dt))
    proj_np = np.dtype(mybir.dt.np(proj_dt))

    x = np.asarray(x, dtype=np.float32)
    edge_index = np.asarray(edge_index)
    edge_attr = np.asarray(edge_attr, dtype=np.float32)
    n, f = x.shape
    assert f == P

    meta, idx16_list, dw_list, ww_list = _preprocess(
        x, edge_index[0], edge_index[1], edge_attr, slots_per_core, msg_np
    )
    n_pad = meta["n_pad"]
    dcore = slots_per_core * P
    wmax = CHUNK_TILES * P

    xp = np.zeros((n_pad, P), dtype=np.float32)
    xp[:n] = x
    xall = np.ascontiguousarray(xp.reshape(n_pad // 2, 2 * P)).astype(msg_np)
    iota = np.tile(
        _enc_bf16(np.arange(wmax)).astype(np.float32)[None, :], (P, 1)
    ).astype(msg_np)

    weights = {
        "wpl": np.ascontiguousarray(np.asarray(w_pos_l, np.float32).T),
        "wpr": np.ascontiguousarray(np.asarray(w_pos_r, np.float32).T),
        "wnl": np.ascontiguousarray(np.asarray(w_neg_l, np.float32).T),
        "wnr": np.ascontiguousarray(np.asarray(w_neg_r, np.float32).T),
    }
    weights = {k: v.astype(proj_np) for k, v in weights.items()}
    bpos = np.asarray(b_pos_r, np.float32).reshape(P, 1)
    bneg = np.asarray(b_neg_r, np.float32).reshape(P, 1)

    nc = _build_program(meta, msg_dt, proj_dt)

    tile_core, tile_slot = meta["tile_core"], meta["tile_slot"]
    xtiles = xp.reshape(-1, P, P)
    in_maps = []
    for c in range(NCORES):
        mytiles = np.zeros((slots_per_core, P, P), dtype=np.float32)
        sel = tile_core == c
        mytiles[tile_slot[sel]] = xtiles[sel]
        xT_c = np.ascontiguousarray(
            mytiles.reshape(dcore, P).T
        ).astype(proj_np)
        in_maps.append(
            dict(
                xall=xall,
                idx16=idx16_list[c], dloc=dw_list[c], wp=ww_list[c],
                dlocn=-dw_list[c], wn=-ww_list[c],
                iota=iota, xT=xT_c,
                bpos=bpos, bneg=bneg, **weights,
            )
        )

    if sim:
        from concourse.bass_interp import MultiCoreSim

        ms = MultiCoreSim(nc, num_cores=NCORES)
        for c in range(NCORES):
            for name, arr in in_maps[c].items():
                ms.cores[c].tensor(name)[:] = arr
        ms.simulate()
        results = [
            {"outT": np.array(ms.cores[c].tensor("outT"))}
            for c in range(NCORES)
        ]
        exec_ns = None
    else:
        br = run_bass_kernel_spmd(
            nc, in_maps, list(range(NCORES)), trace=trace,
            trace_cores=list(range(NCORES)) if (trace and trace_all) else None,
        )
        results = br.results
        exec_ns = br.exec_time_ns

    out = np.empty((n_pad, 2 * P), dtype=np.float32)
    for c in range(NCORES):
        o = results[c]["outT"].T.reshape(slots_per_core, P, 2 * P)
        for k in range(slots_per_core):
            g = np.nonzero((tile_core == c) & (tile_slot == k))[0]
            if g.size:
                out[g[0] * P : g[0] * P + P] = o[k]
    return np.ascontiguousarray(out[:n]), exec_ns


def kernel(**inputs):
    out, _ = _run(**inputs)
    return out



# revision 22
# speedup vs baseline: 1.0001x; 1.0001x over previous
"""Trainium2 Bass kernel for WeightedSignedConv (first_aggr=True) GCN block.

Strategy (8 NeuronCores, one SPMD program):
  - 50000 dst nodes are padded to 50176 = 392 tiles of 128; tiles are
    sorted by edge count and dealt to (core, slot) so all 8 cores see
    nearly identical work per slot (one shared program fits all cores).
  - Host-side: edges are bucketed by (dst chunk of 4 tiles, sign(edge_attr));
    the 1/count normalization is folded into per-edge weights. Within a
    bucket, edges sharing a src PAIR share one gathered slot (pair-dedup,
    ~4% fewer descriptors); same-(pair,parity) extras overflow to their own
    slots. Slots are sorted [even-only | both | odd-only] so two thresholds
    per bucket decide which 128-slot blocks need which parity passes.
  - The gather uses int16 indices (idx = src>>1 < 25088), fetching bf16
    PAIR rows (512B/descriptor) via gpsimd.dma_gather in 1024-idx calls.
    SWDGE descriptor generation (~8ns/idx, Q7-ucode serial) is the
    bottleneck; everything else is hidden under it.
  - Device-side per core: build one-hot scatter matrices
    S[slot, d] = w * (enc(iota) == enc(dloc)) over 512-dst PSUM windows.
    iota/dloc use an injective bf16-exact encoding of [0,512) (identity /
    even / mult-of-4 ranges) since bf16 integers are exact only to 256.
    S-builds are split 3:2 between the Vector engine (fused
    is_equal+mult tensor_scalar) and the Scalar engine (Square+Relu trick:
    w*relu(1-(iota-dloc)^2)), because GPSIMD descriptor generation
    contends with the DVE for an SBUF port and halves its throughput.
  - Aggregation: aggT[f, d] += Xg[slot, f]^T S[slot, d] on the tensor
    engine in PSUM (bf16 operands, fp32 accumulate), reading the even or
    odd half of each gathered pair; wrong-parity weights are 0.
    Projection: out^T[o, d] = W_l^T agg + W_r^T x^T (bf16), finished with
    fused ReLU+bias on the Scalar engine in fp32.
  - Output is produced transposed ([256, D_core] per core); the host
    transposes/reorders, which is pure layout assembly.

Measured: ~830us on 8 trn2 NeuronCores (gather-bound), rel err ~3e-3.
"""

import numpy as np

P = 128
NCORES = 8
CHUNK_TILES = 4          # dst tiles per PSUM window (512 dsts = 1 bank)
MSG_DT_NAME = "bfloat16"  # gathered x + S dtype
PROJ_DT_NAME = "bfloat16"  # projection matmul dtype
GATHER_MAX = 1024        # max idxs per dma_gather call


def _ceil_div(a, b):
    return (a + b - 1) // b


def _enc_bf16(v):
    """Injective map [0,512) -> bf16-exact floats (id, even, mult-of-4)."""
    v = np.asarray(v, dtype=np.float64)
    return np.where(v < 256, v, np.where(v < 384, 2 * v - 256, 4 * v - 1024))


def _preprocess(x, src, dst, attr, slots_per_core, msg_np):
    """Bucket/pad edges; build per-core device arrays + block metadata."""
    n, f = x.shape
    assert f == P
    tiles_total = NCORES * slots_per_core
    n_pad = tiles_total * P

    pos = attr > 0
    neg = attr < 0
    keep = pos | neg
    absa = np.abs(attr)
    cntp = np.bincount(dst[pos], minlength=n).astype(np.float32)
    cntn = np.bincount(dst[neg], minlength=n).astype(np.float32)
    recp = 1.0 / np.maximum(cntp, 1.0)
    recn = 1.0 / np.maximum(cntn, 1.0)
    w1_all = absa.astype(np.float32) * np.where(pos, recp[dst], recn[dst])

    s_ = src[keep].astype(np.int64)
    d_ = dst[keep].astype(np.int64)
    sg = np.where(pos[keep], 0, 1).astype(np.int64)
    w1 = w1_all[keep].astype(np.float32)
    pairidx = s_ >> 1
    parity = s_ & 1

    tile_g = d_ // P

    # Dealing: the program is shared across cores, so each (chunk, sign)
    # bucket is padded to the max slot count over the 8 cores. Try several
    # tile->core dealing strategies and keep the one with the least padding
    # (evaluated on actual deduped slot counts).
    tile_edges = np.bincount(tile_g, minlength=tiles_total)
    tpos = np.bincount(tile_g[sg == 0], minlength=tiles_total)
    tneg = np.bincount(tile_g[sg == 1], minlength=tiles_total)
    order_t = np.argsort(-tile_edges, kind="stable")
    n_chunks_ = _ceil_div(slots_per_core, CHUNK_TILES)

    def _deal_rank():
        rank = np.argsort(np.argsort(-tile_edges))
        return rank % NCORES, rank // NCORES

    def _deal_greedy(per_sign):
        tc = np.empty(tiles_total, dtype=np.int64)
        ts = np.empty(tiles_total, dtype=np.int64)
        cum = np.zeros((NCORES, 2), dtype=np.int64)
        for r in range(slots_per_core):
            if r % CHUNK_TILES == 0:
                cum[:] = 0
            row = order_t[r * NCORES : (r + 1) * NCORES]
            rowtiles = row[np.argsort(-(tpos[row] + tneg[row]),
                                      kind="stable")]
            free = list(range(NCORES))
            for t in rowtiles:
                if per_sign:
                    cost = [max(cum[c, 0] + tpos[t], cum[c, 1] + tneg[t])
                            for c in free]
                else:
                    cost = [cum[c, 0] + cum[c, 1] for c in free]
                c = free.pop(int(np.argmin(cost)))
                tc[t] = c
                ts[t] = r
                cum[c, 0] += tpos[t]
                cum[c, 1] += tneg[t]
        return tc, ts

    def _eval_npad(tc, ts):
        co = tc[tile_g]
        sl = ts[tile_g]
        ch = sl // CHUNK_TILES
        ky = (co * n_chunks_ + ch) * 2 + sg
        nk = NCORES * n_chunks_ * 2
        kq = ky * (1 << 15) + pairidx
        kqs = np.sort(kq)
        newp = np.ones(kqs.size, bool)
        newp[1:] = kqs[1:] != kqs[:-1]
        up = np.bincount((kqs >> 15)[newp], minlength=nk)
        kqp = np.sort(kq * 2 + parity)
        newpp = np.ones(kqp.size, bool)
        newpp[1:] = kqp[1:] != kqp[:-1]
        upp = np.bincount((kqp >> 16)[newpp], minlength=nk)
        epk = np.bincount(ky, minlength=nk)
        spk = (up + (epk - upp)).reshape(NCORES, n_chunks_, 2)
        return int((_ceil_div(spk.max(axis=0), P) * P).sum())

    cands = [_deal_rank(), _deal_greedy(False), _deal_greedy(True)]
    scores = [_eval_npad(tc, ts) for tc, ts in cands]
    tile_core, tile_slot = cands[int(np.argmin(scores))]

    core = tile_core[tile_g]
    slot = tile_slot[tile_g]
    chunk = slot // CHUNK_TILES
    dloc = (slot % CHUNK_TILES) * P + d_ % P

    n_chunks = _ceil_div(slots_per_core, CHUNK_TILES)

    # group key: (core, chunk, sign); within a bucket, edges sharing a
    # src PAIR share one gathered slot (h-passes read even/odd halves), so
    # each pair is fetched once per bucket. Extra edges with the same
    # (pair, parity) overflow into their own slots. Slots are sorted by
    # class [E=h0-only | M=both | O=h1-only] so two thresholds (p0, p1)
    # still decide which blocks need which halves.
    key = (core * n_chunks + chunk) * 2 + sg
    nkeys = NCORES * n_chunks * 2

    ordK = np.lexsort((parity, pairidx, key))
    k_s = key[ordK]
    q_s = pairidx[ordK]
    par_s = parity[ordK]
    dl_s = dloc[ordK]
    w_s = w1[ordK]
    new_pair = np.ones(k_s.size, dtype=bool)
    new_pair[1:] = (k_s[1:] != k_s[:-1]) | (q_s[1:] != q_s[:-1])
    new_pp = new_pair.copy()
    new_pp[1:] |= par_s[1:] != par_s[:-1]
    grp = np.cumsum(new_pair) - 1
    n_grp = int(new_pair.sum())

    g_key = k_s[new_pair]
    g_pair = q_s[new_pair]
    g_d = np.zeros((n_grp, 2), dtype=np.float64)
    g_w = np.zeros((n_grp, 2), dtype=np.float64)
    g_has = np.zeros((n_grp, 2), dtype=bool)
    for h in (0, 1):
        m = new_pp & (par_s == h)
        g_d[grp[m], h] = dl_s[m]
        g_w[grp[m], h] = w_s[m]
        g_has[grp[m], h] = True
    g_class = np.where(g_has[:, 0] & g_has[:, 1], 1,
                       np.where(g_has[:, 0], 0, 2))

    ov = ~new_pp
    o_key = k_s[ov]
    o_pair = q_s[ov]
    o_par = par_s[ov]
    o_d = np.zeros((o_key.size, 2), dtype=np.float64)
    o_w = np.zeros((o_key.size, 2), dtype=np.float64)
    o_d[np.arange(o_key.size), o_par] = dl_s[ov]
    o_w[np.arange(o_key.size), o_par] = w_s[ov]
    o_class = np.where(o_par == 0, 0, 2)

    s_key = np.concatenate([g_key, o_key])
    s_pair = np.concatenate([g_pair, o_pair])
    s_class = np.concatenate([g_class, o_class])
    s_d = np.concatenate([g_d, o_d])
    s_w = np.concatenate([g_w, o_w])

    cnt_kc = np.bincount(s_key * 3 + s_class, minlength=nkeys * 3).reshape(
        NCORES, n_chunks, 2, 3
    )
    nE = cnt_kc[..., 0]
    nM = cnt_kc[..., 1]
    nO = cnt_kc[..., 2]
    tot_kc = nE + nM + nO
    blocks = _ceil_div(tot_kc.max(axis=0), P)  # [chunk, sign]
    blocks = np.maximum(blocks, 1)
    p0 = nE.min(axis=0) // P                   # h1 passes for j >= p0
    p1 = np.minimum(_ceil_div((nE + nM).max(axis=0), P), blocks)

    # layout: per chunk: blocks of sign 0 then sign 1; per block a list of
    # (half, metacol) passes
    gstart = np.zeros((n_chunks, 2), dtype=np.int64)
    chunks = []  # (chunk_idx, width, chunk_block0, nb_chunk)
    windows = {}  # (chunk, sign) -> [(gblock, half, metacol), ...]
    b = 0
    mc = 0
    for c in range(n_chunks):
        cb0 = b
        for s in (0, 1):
            gstart[c, s] = b
            ops = []
            for j in range(int(blocks[c, s])):
                gb = b + j
                if j < p1[c, s]:
                    ops.append((gb, 0, mc))
                    mc += 1
                if j >= p0[c, s]:
                    ops.append((gb, 1, mc))
                    mc += 1
            windows[(c, s)] = ops
            b += int(blocks[c, s])
        w = min(CHUNK_TILES, slots_per_core - c * CHUNK_TILES) * P
        chunks.append((c, w, cb0, b - cb0))
    tot_blocks = b
    tot_cols = mc
    npad = tot_blocks * P

    # per-slot destination position in the padded per-core arrays
    ordS = np.lexsort((s_class, s_key))
    key_s2 = s_key[ordS]
    group_first = np.searchsorted(key_s2, np.arange(nkeys), side="left")
    rank_s = np.arange(key_s2.size) - group_first[key_s2]
    gstart_flat = gstart.reshape(-1)
    local_key = key_s2 % (n_chunks * 2)
    sslot = gstart_flat[local_key] * P + rank_s

    core_s2 = key_s2 // (n_chunks * 2)
    pair_s2 = s_pair[ordS]
    d_s2 = s_d[ordS]
    w_s2 = s_w[ordS]

    # block/half -> metacol lookup
    colmap = -np.ones((tot_blocks, 2), dtype=np.int64)
    for ops in windows.values():
        for gb, h, mcol in ops:
            colmap[gb, h] = mcol

    idx16_list, dw_list, ww_list = [], [], []
    for cc in range(NCORES):
        m = core_s2 == cc
        sp = np.zeros(npad, dtype=np.int64)
        dp = np.zeros((npad, 2), dtype=np.float64)
        wp = np.zeros((npad, 2), dtype=np.float64)
        sp[sslot[m]] = pair_s2[m]
        dp[sslot[m]] = d_s2[m]
        wp[sslot[m]] = w_s2[m]
        tmp = sp.reshape(-1, 16).T.astype(np.int16)
        idx16_list.append(np.tile(tmp, (8, 1)))
        dcols = np.zeros((P, tot_cols), dtype=np.float64)
        wcols = np.zeros((P, tot_cols), dtype=np.float64)
        dp2 = dp.reshape(-1, P, 2)
        wp2 = wp.reshape(-1, P, 2)
        for gb in range(tot_blocks):
            for h in (0, 1):
                mcol = colmap[gb, h]
                if mcol < 0:
                    continue
                dcols[:, mcol] = _enc_bf16(dp2[gb, :, h])
                wcols[:, mcol] = wp2[gb, :, h]
        dw_list.append(np.ascontiguousarray(dcols).astype(np.float32))
        ww_list.append(np.ascontiguousarray(wcols).astype(np.float32))

    m1 = tot_kc[:, :, 1].max(axis=0)  # [chunk] max-core sign-1 slots
    chunk_exact = {}
    for c in range(n_chunks):
        fill = int(m1[c] - (blocks[c, 1] - 1) * P)
        fill = max(1, min(P, fill))
        nb_c = int(blocks[c, 0] + blocks[c, 1])
        chunk_exact[c] = (nb_c - 1) * P + fill

    meta = dict(
        chunk_exact=chunk_exact,
        n=n,
        n_pad=n_pad,
        slots_per_core=slots_per_core,
        n_chunks=n_chunks,
        tot_blocks=tot_blocks,
        tot_cols=tot_cols,
        npad=npad,
        chunks=chunks,
        windows=windows,
        tile_core=tile_core,
        tile_slot=tile_slot,
    )
    return meta, idx16_list, dw_list, ww_list


def _build_program(meta, msg_dt, proj_dt):
    import concourse.bacc as bacc
    import concourse.mybir as mybir
    import concourse.tile as tile

    f32 = mybir.dt.float32
    dcore = meta["slots_per_core"] * P
    wmax = CHUNK_TILES * P
    npairs = meta["n_pad"] // 2

    nc = bacc.Bacc(
        "TRN2", target_bir_lowering=False, debug=False, num_devices=NCORES,
    )
    xall = nc.dram_tensor("xall", [npairs, 2 * P], msg_dt,
                          kind="ExternalInput")
    idx16 = nc.dram_tensor(
        "idx16", [P, meta["npad"] // 16], mybir.dt.int16, kind="ExternalInput"
    )
    dlocd = nc.dram_tensor(
        "dloc", [P, meta["tot_cols"]], f32, kind="ExternalInput"
    )
    wpd = nc.dram_tensor(
        "wp", [P, meta["tot_cols"]], f32, kind="ExternalInput"
    )
    dlocnd = nc.dram_tensor(
        "dlocn", [P, meta["tot_cols"]], f32, kind="ExternalInput"
    )
    wnd = nc.dram_tensor(
        "wn", [P, meta["tot_cols"]], f32, kind="ExternalInput"
    )
    iotad = nc.dram_tensor("iota", [P, wmax], msg_dt, kind="ExternalInput")
    xTd = nc.dram_tensor("xT", [P, dcore], proj_dt, kind="ExternalInput")
    wd = {}
    for nm in ("wpl", "wpr", "wnl", "wnr"):
        wd[nm] = nc.dram_tensor(nm, [P, P], proj_dt, kind="ExternalInput")
    bd = {
        0: nc.dram_tensor("bpos", [P, 1], f32, kind="ExternalInput"),
        1: nc.dram_tensor("bneg", [P, 1], f32, kind="ExternalInput"),
    }
    outd = nc.dram_tensor("outT", [2 * P, dcore], f32, kind="ExternalOutput")

    # process chunks largest-first: the tail after the last gather is the
    # last chunk's compute chain, so make that chunk the smallest
    chunk_order = sorted(meta["chunks"], key=lambda c: -c[3])
    # idx columns for the first-processed chunk load in their own DMA so
    # the first gather doesn't wait on the whole index array
    f_cb0, f_nb = chunk_order[0][2], chunk_order[0][3]
    lo_cols, hi_cols = f_cb0 * 8, (f_cb0 + f_nb) * 8

    with tile.TileContext(nc) as tc:
        with tc.tile_pool(name="const", bufs=1) as cpool, \
             tc.tile_pool(name="work", bufs=4) as wpool, \
             tc.tile_pool(name="spool", bufs=10) as spool, \
             tc.tile_pool(name="psum", bufs=2, space="PSUM") as ppool:
            idx_t = cpool.tile([P, meta["npad"] // 16], mybir.dt.int16)
            dloc_t = cpool.tile([P, meta["tot_cols"]], f32)
            wp_t = cpool.tile([P, meta["tot_cols"]], f32)
            dlocn_t = cpool.tile([P, meta["tot_cols"]], f32)
            wn_t = cpool.tile([P, meta["tot_cols"]], f32)
            iota_t = cpool.tile([P, wmax], msg_dt)
            w_t = {nm: cpool.tile([P, P], proj_dt, name=f"w_{nm}",
                                  tag=f"w_{nm}") for nm in wd}
            b_t = {s: cpool.tile([P, 1], f32, name=f"b_{s}", tag=f"b_{s}")
                   for s in (0, 1)}
            nc.sync.dma_start(out=idx_t[:, lo_cols:hi_cols],
                              in_=idx16[:, lo_cols:hi_cols])
            if lo_cols > 0:
                nc.sync.dma_start(out=idx_t[:, :lo_cols],
                                  in_=idx16[:, :lo_cols])
            if hi_cols < meta["npad"] // 16:
                nc.sync.dma_start(out=idx_t[:, hi_cols:],
                                  in_=idx16[:, hi_cols:])
            nc.sync.dma_start(out=iota_t[:], in_=iotad[:])
            nc.sync.dma_start(out=dloc_t[:], in_=dlocd[:])
            nc.sync.dma_start(out=wp_t[:], in_=wpd[:])
            nc.sync.dma_start(out=dlocn_t[:], in_=dlocnd[:])
            nc.sync.dma_start(out=wn_t[:], in_=wnd[:])
            for nm in wd:
                nc.sync.dma_start(out=w_t[nm][:], in_=wd[nm][:])
            for s in (0, 1):
                nc.sync.dma_start(out=b_t[s][:], in_=bd[s][:])

            wl = {0: w_t["wpl"], 1: w_t["wnl"]}
            wr = {0: w_t["wpr"], 1: w_t["wnr"]}

            spass = 0
            for ci, w, cb0, nb_chunk in chunk_order:
                xg = wpool.tile([P, nb_chunk, 2 * P], msg_dt, name="xg",
                                tag="xg")
                done = 0
                while done < nb_chunk:
                    g = min(nb_chunk - done, GATHER_MAX // P)
                    gb0 = cb0 + done
                    nc.gpsimd.dma_gather(
                        out_ap=xg[:, done : done + g, :],
                        in_ap=xall[:],
                        idxs_ap=idx_t[:, gb0 * 8 : (gb0 + g) * 8],
                        num_idxs=g * P,
                        num_idxs_reg=g * P,
                        elem_size=2 * P,
                        single_packet=False,
                    )
                    done += g

                agg_ps = {
                    s: ppool.tile([P, w], f32, name=f"agg{s}", tag=f"agg{s}")
                    for s in (0, 1)
                }
                for s in (0, 1):
                    ops = meta["windows"][(ci, s)]
                    for j, (gb, h, mcol) in enumerate(ops):
                        s_t = spool.tile([P, w], msg_dt, name="S", tag="S")
                        if spass % 5 < 3:
                            nc.vector.tensor_scalar(
                                out=s_t[:],
                                in0=iota_t[:, :w],
                                scalar1=dloc_t[:, mcol : mcol + 1],
                                scalar2=wp_t[:, mcol : mcol + 1],
                                op0=mybir.AluOpType.is_equal,
                                op1=mybir.AluOpType.mult,
                            )
                        else:
                            z2 = spool.tile([P, w], msg_dt, name="Z", tag="Z")
                            nc.scalar.activation(
                                out=z2[:], in_=iota_t[:, :w],
                                func=mybir.ActivationFunctionType.Square,
                                bias=dlocn_t[:, mcol : mcol + 1],
                            )
                            nc.scalar.activation(
                                out=s_t[:], in_=z2[:],
                                func=mybir.ActivationFunctionType.Relu,
                                scale=wn_t[:, mcol : mcol + 1],
                                bias=wp_t[:, mcol : mcol + 1],
                            )
                        spass += 1
                        nc.tensor.matmul(
                            out=agg_ps[s][:],
                            lhsT=xg[:, gb - cb0, h * P : (h + 1) * P],
                            rhs=s_t[:],
                            start=(j == 0),
                            stop=(j == len(ops) - 1),
                        )

                xT_t = wpool.tile([P, w], proj_dt, name="xT", tag="xT")
                nc.sync.dma_start(
                    out=xT_t[:],
                    in_=xTd[:, ci * wmax : ci * wmax + w],
                )
                for s in (0, 1):
                    agg_sb = wpool.tile([P, w], proj_dt, name=f"aggsb{s}",
                                        tag=f"aggsb{s}")
                    nc.scalar.copy(out=agg_sb[:], in_=agg_ps[s][:])
                    out_ps = ppool.tile([P, w], f32, name=f"out{s}",
                                        tag=f"out{s}")
                    nc.tensor.matmul(
                        out=out_ps[:], lhsT=wl[s][:], rhs=agg_sb[:],
                        start=True, stop=False,
                    )
                    nc.tensor.matmul(
                        out=out_ps[:], lhsT=wr[s][:], rhs=xT_t[:],
                        start=False, stop=True,
                    )
                    out_sb = wpool.tile([P, w], f32, name=f"outsb{s}",
                                        tag=f"outsb{s}")
                    nc.scalar.activation(
                        out=out_sb[:], in_=out_ps[:],
                        func=mybir.ActivationFunctionType.Relu,
                        bias=b_t[s][:],
                    )
                    nc.sync.dma_start(
                        out=outd[s * P : (s + 1) * P,
                                 ci * wmax : ci * wmax + w],
                        in_=out_sb[:],
                    )
    nc.compile()
    return nc


def _run(x, edge_index, edge_attr, w_pos_l, w_pos_r, b_pos_r, w_neg_l,
         w_neg_r, b_neg_r, slots_per_core=49, sim=False, trace=False,
         trace_all=False):
    import concourse.mybir as mybir
    from concourse.bass_utils import run_bass_kernel_spmd

    msg_dt = getattr(mybir.dt, MSG_DT_NAME)
    proj_dt = getattr(mybir.dt, PROJ_DT_NAME)
    msg_np = np.dtype(mybir.dt.np(msg_dt))
    proj_np = np.dtype(mybir.dt.np(proj_dt))

    x = np.asarray(x, dtype=np.float32)
    edge_index = np.asarray(edge_index)
    edge_attr = np.asarray(edge_attr, dtype=np.float32)
    n, f = x.shape
    assert f == P

    meta, idx16_list, dw_list, ww_list = _preprocess(
        x, edge_index[0], edge_index[1], edge_attr, slots_per_core, msg_np
    )
    n_pad = meta["n_pad"]
    dcore = slots_per_core * P
    wmax = CHUNK_TILES * P

    xp = np.zeros((n_pad, P), dtype=np.float32)
    xp[:n] = x
    xall = np.ascontiguousarray(xp.reshape(n_pad // 2, 2 * P)).astype(msg_np)
    iota = np.tile(
        _enc_bf16(np.arange(wmax)).astype(np.float32)[None, :], (P, 1)
    ).astype(msg_np)

    weights = {
        "wpl": np.ascontiguousarray(np.asarray(w_pos_l, np.float32).T),
        "wpr": np.ascontiguousarray(np.asarray(w_pos_r, np.float32).T),
        "wnl": np.ascontiguousarray(np.asarray(w_neg_l, np.float32).T),
        "wnr": np.ascontiguousarray(np.asarray(w_neg_r, np.float32).T),
    }
    weights = {k: v.astype(proj_np) for k, v in weights.items()}
    bpos = np.asarray(b_pos_r, np.float32).reshape(P, 1)
    bneg = np.asarray(b_neg_r, np.float32).reshape(P, 1)

    nc = _build_program(meta, msg_dt, proj_dt)

    tile_core, tile_slot = meta["tile_core"], meta["tile_slot"]
    xtiles = xp.reshape(-1, P, P)
    in_maps = []
    for c in range(NCORES):
        mytiles = np.zeros((slots_per_core, P, P), dtype=np.float32)
        sel = tile_core == c
        mytiles[tile_slot[sel]] = xtiles[sel]
        xT_c = np.ascontiguousarray(
            mytiles.reshape(dcore, P).T
        ).astype(proj_np)
        in_maps.append(
            dict(
                xall=xall,
                idx16=idx16_list[c], dloc=dw_list[c], wp=ww_list[c],
                dlocn=-dw_list[c], wn=-ww_list[c],
                iota=iota, xT=xT_c,
                bpos=bpos, bneg=bneg, **weights,
            )
        )

    if sim:
        from concourse.bass_interp import MultiCoreSim

        ms = MultiCoreSim(nc, num_cores=NCORES)
        for c in range(NCORES):
            for name, arr in in_maps[c].items():
                ms.cores[c].tensor(name)[:] = arr
        ms.simulate()
        results = [
            {"outT": np.array(ms.cores[c].tensor("outT"))}
            for c in range(NCORES)
        ]
        exec_ns = None
    else:
        br = run_bass_kernel_spmd(
            nc, in_maps, list(range(NCORES)), trace=trace,
            trace_cores=list(range(NCORES)) if (trace and trace_all) else None,
        )
        results = br.results
        exec_ns = br.exec_time_ns

    out = np.empty((n_pad, 2 * P), dtype=np.float32)
    for c in range(NCORES):
        o = results[c]["outT"].T.reshape(slots_per_core, P, 2 * P)
        for k in range(slots_per_core):
            g = np.nonzero((tile_core == c) & (tile_slot == k))[0]
            if g.size:
                out[g[0] * P : g[0] * P + P] = o[k]
    return np.ascontiguousarray(out[:n]), exec_ns


def kernel(**inputs):
    out, _ = _run(**inputs)
    return out

